# revision 24
# baseline (speedup 1.0000x reference)
"""BottleNeck-MHSA (B=16, C=512, H=W=32, NH=8, DK=64) on 8 Trainium2 cores.

Sharding: pure data-parallel over batch (2 batches per core), no collectives.

Design (per core, NB=2 batches):
- Augmented-contraction trick: S'^T = [kT|onehots]^T @ [qT|ahT|awT] folds the
  content-dependent rel-pos bias into the energy matmul (K=128 contraction,
  same PE cost as the bare K=64 energy matmul). ahT/awT are content-dependent
  diagonal gathers of rel_h@q / rel_w@q via a DRAM round-trip 3D-strided DMA.
- Softmax skips max-subtraction (logits bounded); exp on ACT with the
  1/sqrt(DK) scale fused; the denominator comes replicated out of the AV
  matmul via ones-columns in the V lhsT; normalization deferred past AV
  (DVE reciprocal+multiply into the O-projection input).
- All SBUF operands bf16 (converted host-side); PSUM fp32. ACT is reserved
  for exp; PSUM evacuations and bf16 copies on DVE (2x/4x perf modes);
  GpSimd only memsets.
- Unified 16-head software pipeline across both batches: per head slot the
  previous head's AV runs as two half-width [128,512] accumulations whose
  normalize fires mid-slot (decouples the next AV from DVE latency), woven
  with this head's S^T/exp at the ACT exp cadence, plus gated filler chunks:
  batch 1's projections and both batches' O-projections, split into ~2-4
  matmul closures. Gates keep PE emission topological (FIFO-safe).
- PSUM (8 banks): pst 2x[128,1024] (4) + proj 2x[128,512] (2) +
  av 2x[128,512] (2).
- TimelineSim: ~194.6 us/core (PE-busy floor ~171 us); rel err ~5.3e-3.
"""

from contextlib import ExitStack

import numpy as np

import concourse.bass as bass
import concourse.tile as tile
from concourse import bacc, mybir
from concourse.ap import AP
from concourse.bass_utils import run_bass_kernel_spmd

FP32 = mybir.dt.float32
BF16 = mybir.dt.bfloat16
Exp = mybir.ActivationFunctionType.Exp

B = 16
C = 512
N = 1024
NH = 8
DK = 64
HW = 32
NCORES = 8
NB = B // NCORES  # batches per core


def _build_body(ctx: ExitStack, tc: tile.TileContext, outs, ins, NB: int):
    nc = tc.nc
    x_in, wq_in, wk_in, wv_in, wo_in, oh_in, rh_in, rw_in, bo_in, id_in = ins
    y_out = outs[0]

    consts = ctx.enter_context(tc.tile_pool(name="consts", bufs=1))
    dbl = ctx.enter_context(tc.tile_pool(name="dbl", bufs=2))
    work = ctx.enter_context(tc.tile_pool(name="work", bufs=2))
    # est tiles: two full heads of 8 tiles each — AV(h) reads all 8 of head
    # h's tiles in every qb-phase of slot h+1, while S/exp(h+1) writes its 8.
    expp = ctx.enter_context(tc.tile_pool(name="expp", bufs=16))
    pstp = ctx.enter_context(tc.tile_pool(name="pstp", bufs=2, space="PSUM"))
    mmp = ctx.enter_context(tc.tile_pool(name="mmp", bufs=2, space="PSUM"))
    avp = ctx.enter_context(tc.tile_pool(name="avp", bufs=2, space="PSUM"))
    dram = ctx.enter_context(tc.tile_pool(name="dram", bufs=2, space="DRAM"))

    # ---------------- tiles ----------------
    # weight layout: [128 (cin within kc-block), (kc, cout_mc)]; slice for
    # (kc, mc) = cols [kc*512 + mc*128, +128)
    w_t = {nm: consts.tile([128, 4 * C], BF16, tag=nm, name=nm) for nm in ("wq", "wk", "wv", "wo")}

    def wsl(nm, kc, mc):
        return w_t[nm][:, kc * 512 + mc * 128 : kc * 512 + (mc + 1) * 128]

    oh_t = consts.tile([64, N], BF16, tag="onehot", name="onehot")
    rh_t = consts.tile([128, 126], BF16, tag="relh2", name="relh2")
    rw_t = consts.tile([128, 126], BF16, tag="relw2", name="relw2")
    bo_t = consts.tile([128, 4], FP32, tag="bo", name="bo")
    id_t = consts.tile([128, 128], BF16, tag="ident", name="ident")

    bt = {}
    for b in range(NB):
        bt[b] = dict(
            # x layout: [128 (cin within kc), (kc, n)]
            x=dbl.tile([128, 4 * N], BF16, tag="x", name=f"x_{b}"),
            q=[dbl.tile([128, N], BF16, tag=f"qaug{h}", name=f"qaug{h}_{b}") for h in range(NH)],
            k=[dbl.tile([128, N], BF16, tag=f"kaug{h}", name=f"kaug{h}_{b}") for h in range(NH)],
            # vaug[nb]: AV moving operand; head h at cols [65h,65h+64)=V, col 65h+64=ones
            # (denominator column for the transposed AV)
            v=[dbl.tile([128, 520], BF16, tag=f"vaug{nb}", name=f"vaug{nb}_{b}") for nb in range(8)],
            o=[dbl.tile([128, N], BF16, tag=f"oin{kc}", name=f"oin{kc}_{b}") for kc in range(4)],
            # ot[qq]: normalized attention out per qb-quad,
            # [128 pix, (4 qb, 8 h, 64 d)] bf16
            ot=[dbl.tile([128, 2048], BF16, tag=f"ot{qq}", name=f"ot{qq}_{b}") for qq in range(2)],
        )

    def xs(b, kc, lo, hi):
        return bt[b]["x"][:, kc * N + lo : kc * N + hi]

    def emit_x_load(b, cold=False):
        if not cold:
            for kc in range(4):
                nc.sync.dma_start(
                    bt[b]["x"][:, kc * N : (kc + 1) * N],
                    x_in[b, kc * 128 : (kc + 1) * 128, :],
                )
            return
        # cold start: (wq slice, x chunk) pairs so each kc matmul's inputs
        # arrive together; nn0 halves first to match matmul order
        for kc in range(4):
            nc.sync.dma_start(
                w_t["wq"][:, kc * 512 : (kc + 1) * 512],
                wq_in[:, kc * 512 : (kc + 1) * 512],
            )
            nc.sync.dma_start(
                bt[b]["x"][:, kc * N : kc * N + 512],
                x_in[b, kc * 128 : (kc + 1) * 128, 0:512],
            )
        for kc in range(4):
            nc.sync.dma_start(
                bt[b]["x"][:, kc * N + 512 : kc * N + 1024],
                x_in[b, kc * 128 : (kc + 1) * 128, 512:1024],
            )

    def emit_vaug_ones(b):
        for nb in range(8):
            va = bt[b]["v"][nb][:]
            ones_ap = AP(va.tensor, va.offset + 64, [[520, 128], [65, 8]])
            nc.gpsimd.memset(ones_ap, 1.0)

    # ---- projection chunk builders (each chunk ~1-4 matmuls + evac work) ----
    def proj_chunks(b):
        """List of (gate_slot, closure); call in list order (dep-ordered)."""
        qaug, kaug, vaug = bt[b]["q"], bt[b]["k"], bt[b]["v"]
        state = {}

        def qk_full(mc, nm):
            # batch-0 only: full-width [128,1024] psum from the (idle) pst
            # pool; evacuations split DVE/ACT (ACT idle before attention)
            def f():
                p = pstp.tile([128, N], FP32, tag="pst", name="pst")
                for nn in range(2):
                    for kc in range(4):
                        nc.tensor.matmul(
                            p[:, nn * 512 : (nn + 1) * 512],
                            wsl(nm, kc, mc),
                            xs(b, kc, nn * 512, (nn + 1) * 512),
                            start=(kc == 0),
                            stop=(kc == 3),
                        )
                if nm == "wq":
                    qpair = work.tile([128, N], BF16, tag="qpair", name="qpair", bufs=2)
                    state["qpair", mc] = qpair
                    nc.vector.tensor_copy(qpair[:, 0:512], p[:, 0:512])
                    nc.scalar.copy(qpair[:, 512:1024], p[:, 512:1024])
                else:
                    nc.vector.tensor_copy(kaug[2 * mc][0:64, :], p[0:64, :])
                    nc.scalar.copy(kaug[2 * mc + 1][0:64, :], p[64:128, :])
                    nc.vector.tensor_copy(kaug[2 * mc][64:128, :], oh_t[:])
                    nc.vector.tensor_copy(kaug[2 * mc + 1][64:128, :], oh_t[:])
            return f

        def qk_half(mc, nm, nn):
            def f():
                p = mmp.tile([128, 512], FP32, tag="mm", name="mm")
                for kc in range(4):
                    nc.tensor.matmul(
                        p[:],
                        wsl(nm, kc, mc),
                        xs(b, kc, nn * 512, (nn + 1) * 512),
                        start=(kc == 0),
                        stop=(kc == 3),
                    )
                if nm == "wq":
                    qpair = state.get(("qpair", mc))
                    if qpair is None:
                        qpair = work.tile([128, N], BF16, tag="qpair", name="qpair", bufs=2)
                        state["qpair", mc] = qpair
                    nc.vector.tensor_copy(qpair[:, nn * 512 : (nn + 1) * 512], p[:])
                else:
                    sl = slice(nn * 512, (nn + 1) * 512)
                    nc.vector.tensor_copy(kaug[2 * mc][0:64, sl], p[0:64, :])
                    nc.vector.tensor_copy(kaug[2 * mc + 1][0:64, sl], p[64:128, :])
                    if nn == 1:
                        nc.gpsimd.tensor_copy(kaug[2 * mc][64:128, :], oh_t[:])
                        nc.gpsimd.tensor_copy(kaug[2 * mc + 1][64:128, :], oh_t[:])
            return f

        def rel_half(mc, which, nn):
            def f():
                qpair = state["qpair", mc]
                p = mmp.tile([126, 512], FP32, tag="mm", name="mm")
                if which == "lh":
                    nc.tensor.matmul(
                        p[:],
                        rh_t[:],
                        qpair[:, nn * 512 : (nn + 1) * 512],
                        start=True,
                        stop=True,
                    )
                else:
                    qp = qpair[:]
                    ym = AP(qp.tensor, qp.offset + nn * 16, [[1024, 128], [1, 16], [32, 32]])
                    nc.tensor.matmul(p[:], rw_t[:], ym, start=True, stop=True)
                key = ("l" + which[1], mc)
                lt = state.get(key)
                if lt is None:
                    lt = work.tile([126, N], BF16, tag=f"l{which[1]}t", name=f"l{which[1]}t", bufs=2)
                    state[key] = lt
                    state[key + ("d",)] = dram.tile([126, N], BF16, tag=f"l{which[1]}td", name=f"l{which[1]}td")
                if b == 0 and nn == 1:
                    # batch-0 serial phase: ACT is idle, DVE is the pacer
                    nc.scalar.copy(lt[:, 512:1024], p[:])
                else:
                    nc.vector.tensor_copy(lt[:, nn * 512 : (nn + 1) * 512], p[:])
                if nn == 1:
                    nc.sync.dma_start(state[key + ("d",)][:], lt[:])
            return f

        def v_block(nb):
            def f():
                pv = mmp.tile([128, 512], FP32, tag="mm", name="mm")
                for kc in range(4):
                    nc.tensor.matmul(
                        pv[:],
                        xs(b, kc, nb * 128, (nb + 1) * 128),
                        w_t["wv"][:, kc * 512 : (kc + 1) * 512],
                        start=(kc == 0),
                        stop=(kc == 3),
                    )
                va = vaug[nb][:]
                vdst = AP(va.tensor, va.offset, [[520, 128], [65, 8], [1, 64]])
                pvs = pv[:]
                vsrc = AP(pvs.tensor, pvs.offset, [[512, 128], [64, 8], [1, 64]])
                nc.vector.tensor_copy(vdst, vsrc)
            return f

        def gathers(mc, hh):
            def f():
                # b1's SBUF->SBUF copies go on the (otherwise idle) GpSimd
                # engine to keep DVE headroom during the attention slots.
                cpy = nc.gpsimd.tensor_copy if b == 1 else nc.vector.tensor_copy
                h = 2 * mc + hh
                qpair = state["qpair", mc]
                cpy(qaug[h][0:64, :], qpair[hh * 64 : hh * 64 + 64, :])
                lhd = state["lh", mc, "d"][:]
                diag_h = AP(
                    lhd.tensor,
                    lhd.offset + (hh * 63 + 31) * N,
                    [[N, 32], [-(N - 32), 32], [1, 32]],
                )
                nc.sync.dma_start(
                    qaug[h][64:96, :].rearrange("p (a b) -> p a b", a=32), diag_h
                )
                awym = work.tile([32, N], BF16, tag="awym", name="awym")
                lwd = state["lw", mc, "d"][:]
                diag_w = AP(
                    lwd.tensor,
                    lwd.offset + (hh * 63 + 31) * N,
                    [[N, 32], [-(N - 32), 32], [1, 32]],
                )
                nc.sync.dma_start(
                    awym[:].rearrange("p (a b) -> p a b", a=32), diag_w
                )
                aw = awym[:]
                src = AP(aw.tensor, aw.offset, [[1024, 32], [1, 32], [32, 32]])
                cpy(
                    qaug[h][96:128, :].rearrange("p (a b) -> p a b", a=32), src
                )
            return f

        def tail_chunks(mc):
            return [
                rel_half(mc, "lh", 0),
                rel_half(mc, "lh", 1),
                rel_half(mc, "lw", 0),
                rel_half(mc, "lw", 1),
                gathers(mc, 0),
                gathers(mc, 1),
                v_block(2 * mc),
                v_block(2 * mc + 1),
            ]

        if b == 0:
            qks = [qk_full(mc, nm) for mc in range(4) for nm in ("wq", "wk")]
            tails = [f for mc in range(4) for f in tail_chunks(mc)]
            return qks, tails
        # batch 1, woven into attention slots at 4 chunks/slot. Order is
        # chosen so every chunk is EMITTED before the attention stream
        # reads its outputs (deadlines: vaug(*) before slot 9 = AV(1,0);
        # qaug/kaug(1,h) before slot 8+h). v_blocks depend only on x+wv,
        # so they are hoisted ahead of mc2/mc3.
        def qk_rel_g(mc):
            return [
                qk_half(mc, "wq", 0), qk_half(mc, "wq", 1),
                qk_half(mc, "wk", 0), qk_half(mc, "wk", 1),
                rel_half(mc, "lh", 0), rel_half(mc, "lh", 1),
                rel_half(mc, "lw", 0), rel_half(mc, "lw", 1),
                gathers(mc, 0), gathers(mc, 1),
            ]

        # gates spread the b1 projections across slots 1-12 so their DVE
        # evacs never cluster enough to back up the AV normalizes
        out = [(1, f) for f in qk_rel_g(0)]
        out += [(3, f) for f in qk_rel_g(1)]
        out += [(4, v_block(0)), (4, v_block(1))]
        out += [(5, v_block(nb)) for nb in range(2, 8)]
        out += [(8, f) for f in qk_rel_g(2)]
        out += [(11, f) for f in qk_rel_g(3)]
        return out


    def oproj_chunks(b):
        """O-projection. Batch 0: whole chunks gated after finish_av(0,7)
        (slot 9). Batch 1 necessarily tails the pipeline: the kc0-2
        accumulation (needs heads 0-5 only) runs from slot 15 / the av-tail,
        and only the kc3+bias+store part waits for the final normalize;
        bias adds go on ACT (idle at drain, Identity shares the exp set)."""
        oin = bt[b]["o"]
        chunks = []

        def po_full(mc, nn, oo):
            def f():
                po = mmp.tile([128, 512], FP32, tag="mm", name="mm")
                for kc in range(4):
                    nc.tensor.matmul(
                        po[:],
                        wsl("wo", kc, mc),
                        oin[kc][:, nn * 512 : (nn + 1) * 512],
                        start=(kc == 0),
                        stop=(kc == 3),
                    )
                nc.vector.tensor_add(
                    oo[:, nn * 512 : (nn + 1) * 512],
                    po[:],
                    bo_t[:, mc : mc + 1].broadcast_to((128, 512)),
                )
                nc.sync.dma_start(
                    y_out[b, mc * 128 : (mc + 1) * 128, nn * 512 : (nn + 1) * 512],
                    oo[:, nn * 512 : (nn + 1) * 512],
                )
            return f

        def po_partA(mc, st):
            # b1: kc0-2 accumulation for both halves in one pst-pool tile —
            # the pst pool is idle once the last S head is done (slot 15).
            def f():
                po = pstp.tile([128, N], FP32, tag="pst", name="pst")
                st["po", mc] = po
                for nn in range(2):
                    for kc in range(3):
                        nc.tensor.matmul(
                            po[:, nn * 512 : (nn + 1) * 512],
                            wsl("wo", kc, mc),
                            oin[kc][:, nn * 512 : (nn + 1) * 512],
                            start=(kc == 0),
                            stop=False,
                        )
            return f

        def po_partB(mc, oo, st):
            # final kc3 + bias on ACT (idle at drain) + store
            def f():
                po = st["po", mc]
                for nn in range(2):
                    nc.tensor.matmul(
                        po[:, nn * 512 : (nn + 1) * 512],
                        wsl("wo", 3, mc),
                        oin[3][:, nn * 512 : (nn + 1) * 512],
                        start=False,
                        stop=True,
                    )
                for nn in range(2):
                    nc.scalar.activation(
                        oo[:, nn * 512 : (nn + 1) * 512],
                        po[:, nn * 512 : (nn + 1) * 512],
                        mybir.ActivationFunctionType.Identity,
                        bias=bo_t[:, mc : mc + 1],
                        scale=1.0,
                    )
                    nc.sync.dma_start(
                        y_out[b, mc * 128 : (mc + 1) * 128, nn * 512 : (nn + 1) * 512],
                        oo[:, nn * 512 : (nn + 1) * 512],
                    )
            return f

        if b == 0:
            for mc in range(4):
                oo = work.tile([128, N], FP32, tag="oout", name="oout", bufs=2)
                chunks.append((10, po_full(mc, 0, oo)))
                chunks.append((10, po_full(mc, 1, oo)))
            return chunks
        st = {}
        oos = {mc: work.tile([128, N], FP32, tag="oout", name="oout", bufs=2) for mc in range(4)}
        chunks.append((16, po_partA(0, st)))
        chunks.append((16, po_partA(1, st)))
        for mc in range(4):
            chunks.append((17, po_partB(mc, oos[mc], st)))
            if mc + 2 < 4:
                chunks.append((17, po_partA(mc + 2, st)))
        return chunks

    def proj_chunks0_parts():
        return proj_chunks(0)

    # =================== emission ===================
    emit_x_load(0, cold=True)
    for kc in range(4):
        nc.sync.dma_start(
            w_t["wk"][:, kc * 512 : (kc + 1) * 512], wk_in[:, kc * 512 : (kc + 1) * 512]
        )
    for kc in range(4):
        nc.sync.dma_start(
            w_t["wv"][:, kc * 512 : (kc + 1) * 512], wv_in[:, kc * 512 : (kc + 1) * 512]
        )
    nc.sync.dma_start(oh_t[:], oh_in[:])
    nc.sync.dma_start(rh_t[:], rh_in[:])
    nc.sync.dma_start(rw_t[:], rw_in[:])
    nc.sync.dma_start(bo_t[:], bo_in[:].rearrange("(c p) one -> p (c one)", p=128))
    nc.sync.dma_start(id_t[:], id_in[:])

    # pre-warm the ACT exp table during the projection phase
    warm = work.tile([128, 4], FP32, tag="warm", name="warm", bufs=1)
    nc.scalar.activation(warm[:], bo_t[:], Exp, bias=0.0, scale=1.0)

    # batch 0 projections: coarse software pipeline, one mc ahead (qk of
    # mc+1 before tail of mc so PE isn't gated on tail's DVE evacs)
    g0, t0 = proj_chunks0_parts()
    order = g0[0:4] + t0[0:8] + g0[4:8] + t0[8:32]
    for ci, f in enumerate(order):
        f()
        if ci == 5:
            emit_vaug_ones(0)

    emit_x_load(1)
    nc.sync.dma_start(w_t["wo"][:], wo_in[:])
    emit_vaug_ones(1)

    filler = proj_chunks(1) + oproj_chunks(0) + oproj_chunks(1)

    

    def fill(slot, k=1):
        while k > 0 and filler and filler[0][0] <= slot:
            filler.pop(0)[1]()
            k -= 1

    # ---- unified attention pipeline: 16 head slots across both batches ----
    heads = [(b, h) for b in range(NB) for h in range(NH)]
    est_all = {}

    def emit_st_jb(bh, jb):
        b, h = bh
        pst = pstp.tile([128, N], FP32, tag="pst", name="pst")
        for nn in range(2):
            nc.tensor.matmul(
                pst[:, nn * 512 : (nn + 1) * 512],
                bt[b]["k"][h][:, jb * 128 : (jb + 1) * 128],
                bt[b]["q"][h][:, nn * 512 : (nn + 1) * 512],
                start=True,
                stop=True,
            )
        nc.scalar.activation(est_all[bh][jb][:], pst[:], Exp, bias=0.0, scale=0.125)

    qstate = {}

    def emit_av_qb(bh, qb):
        # Transposed AV: est chunks are the stationary operand (weight loads
        # are free on PE); the moving operand is V+ones (65 cols) instead of
        # est (1024 cols) -> 65*8 instead of 1024*8 PE cycles per (head, qb).
        # Four qb's share one [128, 4*65] PSUM tile (one bank): one batched
        # recip+mult per quad keeps the DVE work low and gives the av-tag
        # rotation ~4 phases of slack before the PE would block on it.
        b, h = bh
        qq, lo = divmod(qb, 4)
        if lo == 0:
            qstate[bh, qq] = avp.tile([128, 260], FP32, tag="av", name="pav")
        pav = qstate[bh, qq]
        for kb in range(8):
            nc.tensor.matmul(
                pav[:, lo * 65 : lo * 65 + 65],
                est_all[bh][kb][:, qb * 128 : (qb + 1) * 128],
                bt[b]["v"][kb][:, h * 65 : h * 65 + 65],
                start=(kb == 0),
                stop=(kb == 7),
            )
        if lo == 3:
            del qstate[bh, qq]
            pv = pav[:]
            rec = work.tile([128, 4], FP32, tag="rec", name="rec", bufs=4)
            nc.vector.reciprocal(
                rec[:], AP(pv.tensor, pv.offset + 64, [[260, 128], [65, 4]])
            )
            ot = bt[b]["ot"][qq][:]
            nc.vector.tensor_mul(
                AP(ot.tensor, ot.offset + h * 64, [[2048, 128], [512, 4], [1, 64]]),
                AP(pv.tensor, pv.offset, [[260, 128], [65, 4], [1, 64]]),
                rec[:].broadcast_to((128, 4, 64)),
            )

    def emit_tr(b, qb, kc):
        # PE transpose (via identity) of the normalized [pix, hd] chunk back
        # to [hd, pix] for the O-projection; shares the "av" PSUM tag.
        ptr = avp.tile([128, 128], BF16, tag="av", name="ptr")
        qq, lo = divmod(qb, 4)
        nc.tensor.transpose(
            ptr[:], bt[b]["ot"][qq][:, lo * 512 + kc * 128 : lo * 512 + (kc + 1) * 128], id_t[:]
        )
        nc.vector.tensor_copy(bt[b]["o"][kc][:, qb * 128 : (qb + 1) * 128], ptr[:])

    tr_queue = []

    def emit_slot(i, bh, prev):
        if bh is not None:
            est_all[bh] = [expp.tile([128, N], BF16, tag="expst", name="expst") for _ in range(8)]
        for k in range(8):
            if bh is not None:
                emit_st_jb(bh, k)
            if prev is not None:
                emit_av_qb(prev, k)
            if tr_queue:
                emit_tr(*tr_queue.pop(0))
            fill(i)
        # transposes for this slot's (odd) head trail by one slot: the quad
        # norms complete at phases 3/7, so trs can't interleave same-slot.
        if prev is not None and prev[1] % 2 == 1:
            tr_queue.extend((prev[0], qb, prev[1] // 2) for qb in range(8))
        fill(i, k=2)

    for i, bh in enumerate(heads):
        emit_slot(i, bh, heads[i - 1] if i > 0 else None)
    # tail: AV of the last head + its transposes + remaining fillers
    emit_slot(16, None, heads[-1])
    while tr_queue:
        emit_tr(*tr_queue.pop(0))
    fill(99, k=len(filler))


def _host_prep(w_q, w_k, w_v, w_o, b_o, rel_h, rel_w):
    perm = np.array([(c % 64) * 8 + c // 64 for c in range(C)])  # c' -> orig c
    oh = np.zeros((64, N), np.float32)
    j = np.arange(N)
    oh[j // HW, j] = 1.0
    oh[32 + j % HW, j] = 1.0
    rh2 = np.zeros((128, 126), np.float32)
    rh2[0:64, 0:63] = rel_h.T
    rh2[64:128, 63:126] = rel_h.T
    rw2 = np.zeros((128, 126), np.float32)
    rw2[0:64, 0:63] = rel_w.T
    rw2[64:128, 63:126] = rel_w.T
    import ml_dtypes

    bf = lambda a: np.ascontiguousarray(a).astype(ml_dtypes.bfloat16)

    def wpack(w):  # (C_in, C_out) -> (128, (kc, cout))
        return w.reshape(4, 128, C).transpose(1, 0, 2).reshape(128, 4 * C)

    return dict(
        wq=bf(wpack(w_q[perm, :].T)),
        wk=bf(wpack(w_k[perm, :].T)),
        wv=bf(wpack(w_v[perm, :].T)),
        wo=bf(wpack(w_o.T)),
        onehot=bf(oh),
        relh2=bf(rh2),
        relw2=bf(rw2),
        bo=np.ascontiguousarray(b_o.reshape(C, 1), dtype=np.float32),
        ident=bf(np.eye(128, dtype=np.float32)),
    )


_CACHE = {}


def _build_program():
    if "nc" in _CACHE:
        return _CACHE["nc"], _CACHE["names"]
    nc = bacc.Bacc("TRN2", target_bir_lowering=False, debug=False, num_devices=NCORES)
    specs = [
        ("x", (NB, C, N), BF16),
        ("wq", (128, 4 * C), BF16),
        ("wk", (128, 4 * C), BF16),
        ("wv", (128, 4 * C), BF16),
        ("wo", (128, 4 * C), BF16),
        ("onehot", (64, N), BF16),
        ("relh2", (128, 126), BF16),
        ("relw2", (128, 126), BF16),
        ("bo", (C, 1), FP32),
        ("ident", (128, 128), BF16),
    ]
    in_aps = [nc.dram_tensor(nm, list(shape), dt, kind="ExternalInput").ap() for nm, shape, dt in specs]
    out_ap = nc.dram_tensor("y", [NB, C, N], FP32, kind="ExternalOutput").ap()
    with tile.TileContext(nc) as tc:
        with ExitStack() as ctx:
            _build_body(ctx, tc, [out_ap], in_aps, NB)
    nc.compile()
    _CACHE["nc"] = nc
    _CACHE["names"] = [s[0] for s in specs]
    return nc, _CACHE["names"]


def _run(inputs, trace=False, tmpdir=None):
    import ml_dtypes

    x = np.asarray(inputs["x"], dtype=np.float32)
    cst = _host_prep(
        np.asarray(inputs["w_q"], np.float32),
        np.asarray(inputs["w_k"], np.float32),
        np.asarray(inputs["w_v"], np.float32),
        np.asarray(inputs["w_o"], np.float32),
        np.asarray(inputs["b_o"], np.float32),
        np.asarray(inputs["rel_h"], np.float32),
        np.asarray(inputs["rel_w"], np.float32),
    )
    nc, _ = _build_program()
    xb = np.ascontiguousarray(x.reshape(B, C, N)).astype(ml_dtypes.bfloat16)
    in_maps = []
    for c in range(NCORES):
        m = dict(cst)
        m["x"] = np.ascontiguousarray(xb[c * NB : (c + 1) * NB])
        in_maps.append(m)
    res = run_bass_kernel_spmd(
        nc, in_maps, core_ids=list(range(NCORES)), trace=trace, tmpdir=tmpdir
    )
    out = np.empty((B, C, HW, HW), np.float32)
    for c in range(NCORES):
        out[c * NB : (c + 1) * NB] = res.results[c]["y"].reshape(NB, C, HW, HW)
    return out, res


def kernel(**inputs):
    out, _ = _run(inputs, trace=False)
    return out



# revision 26
# speedup vs baseline: 1.0039x; 1.0039x over previous
"""BottleNeck-MHSA (B=16, C=512, H=W=32, NH=8, DK=64) on 8 Trainium2 cores.

Sharding: pure data-parallel over batch (2 batches per core), no collectives.

Design (per core, NB=2 batches):
- Augmented-contraction trick: S'^T = [kT|onehots]^T @ [qT|ahT|awT] folds the
  content-dependent rel-pos bias into the energy matmul (K=128 contraction,
  same PE cost as the bare K=64 energy matmul). ahT/awT are content-dependent
  diagonal gathers of rel_h@q / rel_w@q via a DRAM round-trip 3D-strided DMA.
- Softmax skips max-subtraction (logits bounded); exp on ACT with the
  1/sqrt(DK) scale fused; the denominator comes replicated out of the AV
  matmul via ones-columns in the V lhsT; normalization deferred past AV
  (DVE reciprocal+multiply into the O-projection input).
- All SBUF operands bf16 (converted host-side); PSUM fp32. ACT is reserved
  for exp; PSUM evacuations and bf16 copies on DVE (2x/4x perf modes);
  GpSimd only memsets.
- Unified 16-head software pipeline across both batches: per head slot the
  previous head's AV runs as two half-width [128,512] accumulations whose
  normalize fires mid-slot (decouples the next AV from DVE latency), woven
  with this head's S^T/exp at the ACT exp cadence, plus gated filler chunks:
  batch 1's projections and both batches' O-projections, split into ~2-4
  matmul closures. Gates keep PE emission topological (FIFO-safe).
- PSUM (8 banks): pst 2x[128,1024] (4) + proj 2x[128,512] (2) +
  av 2x[128,512] (2).
- TimelineSim: ~194.6 us/core (PE-busy floor ~171 us); rel err ~5.3e-3.
"""

from contextlib import ExitStack

import numpy as np

import concourse.bass as bass
import concourse.tile as tile
from concourse import bacc, mybir
from concourse.ap import AP
from concourse.bass_utils import run_bass_kernel_spmd

FP32 = mybir.dt.float32
BF16 = mybir.dt.bfloat16
Exp = mybir.ActivationFunctionType.Exp

B = 16
C = 512
N = 1024
NH = 8
DK = 64
HW = 32
NCORES = 8
NB = B // NCORES  # batches per core


def _build_body(ctx: ExitStack, tc: tile.TileContext, outs, ins, NB: int):
    nc = tc.nc
    x_in, wq_in, wk_in, wv_in, wo_in, oh_in, rh_in, rw_in, bo_in, id_in = ins
    y_out = outs[0]

    consts = ctx.enter_context(tc.tile_pool(name="consts", bufs=1))
    dbl = ctx.enter_context(tc.tile_pool(name="dbl", bufs=2))
    work = ctx.enter_context(tc.tile_pool(name="work", bufs=2))
    # est tiles: two full heads of 8 tiles each — AV(h) reads all 8 of head
    # h's tiles in every qb-phase of slot h+1, while S/exp(h+1) writes its 8.
    expp = ctx.enter_context(tc.tile_pool(name="expp", bufs=16))
    pstp = ctx.enter_context(tc.tile_pool(name="pstp", bufs=2, space="PSUM"))
    mmp = ctx.enter_context(tc.tile_pool(name="mmp", bufs=2, space="PSUM"))
    avp = ctx.enter_context(tc.tile_pool(name="avp", bufs=2, space="PSUM"))
    dram = ctx.enter_context(tc.tile_pool(name="dram", bufs=2, space="DRAM"))

    # ---------------- tiles ----------------
    # weight layout: [128 (cin within kc-block), (kc, cout_mc)]; slice for
    # (kc, mc) = cols [kc*512 + mc*128, +128)
    w_t = {nm: consts.tile([128, 4 * C], BF16, tag=nm, name=nm) for nm in ("wq", "wk", "wv", "wo")}

    def wsl(nm, kc, mc):
        return w_t[nm][:, kc * 512 + mc * 128 : kc * 512 + (mc + 1) * 128]

    oh_t = consts.tile([64, N], BF16, tag="onehot", name="onehot")
    rh_t = consts.tile([128, 126], BF16, tag="relh2", name="relh2")
    rw_t = consts.tile([128, 126], BF16, tag="relw2", name="relw2")
    bo_t = consts.tile([128, 4], FP32, tag="bo", name="bo")
    id_t = consts.tile([128, 128], BF16, tag="ident", name="ident")

    bt = {}
    for b in range(NB):
        bt[b] = dict(
            # x layout: [128 (cin within kc), (kc, n)]
            x=dbl.tile([128, 4 * N], BF16, tag="x", name=f"x_{b}"),
            q=[dbl.tile([128, N], BF16, tag=f"qaug{h}", name=f"qaug{h}_{b}") for h in range(NH)],
            k=[dbl.tile([128, N], BF16, tag=f"kaug{h}", name=f"kaug{h}_{b}") for h in range(NH)],
            # vaug[nb]: AV moving operand; head h at cols [65h,65h+64)=V, col 65h+64=ones
            # (denominator column for the transposed AV)
            v=[dbl.tile([128, 520], BF16, tag=f"vaug{nb}", name=f"vaug{nb}_{b}") for nb in range(8)],
            o=[dbl.tile([128, N], BF16, tag=f"oin{kc}", name=f"oin{kc}_{b}") for kc in range(4)],
            # ot[qq]: normalized attention out per qb-quad,
            # [128 pix, (4 qb, 8 h, 64 d)] bf16
            ot=[dbl.tile([128, 2048], BF16, tag=f"ot{qq}", name=f"ot{qq}_{b}") for qq in range(2)],
        )

    def xs(b, kc, lo, hi):
        return bt[b]["x"][:, kc * N + lo : kc * N + hi]

    def emit_x_load(b, cold=False):
        if not cold:
            for kc in range(4):
                nc.sync.dma_start(
                    bt[b]["x"][:, kc * N : (kc + 1) * N],
                    x_in[b, kc * 128 : (kc + 1) * 128, :],
                )
            return
        # cold start: (wq slice, x chunk) pairs so each kc matmul's inputs
        # arrive together; nn0 halves first to match matmul order
        for kc in range(4):
            nc.sync.dma_start(
                w_t["wq"][:, kc * 512 : (kc + 1) * 512],
                wq_in[:, kc * 512 : (kc + 1) * 512],
            )
            nc.sync.dma_start(
                bt[b]["x"][:, kc * N : kc * N + 512],
                x_in[b, kc * 128 : (kc + 1) * 128, 0:512],
            )
        for kc in range(4):
            nc.sync.dma_start(
                bt[b]["x"][:, kc * N + 512 : kc * N + 1024],
                x_in[b, kc * 128 : (kc + 1) * 128, 512:1024],
            )

    def emit_vaug_ones(b):
        for nb in range(8):
            va = bt[b]["v"][nb][:]
            ones_ap = AP(va.tensor, va.offset + 64, [[520, 128], [65, 8]])
            nc.gpsimd.memset(ones_ap, 1.0)

    # ---- projection chunk builders (each chunk ~1-4 matmuls + evac work) ----
    def proj_chunks(b):
        """List of (gate_slot, closure); call in list order (dep-ordered)."""
        qaug, kaug, vaug = bt[b]["q"], bt[b]["k"], bt[b]["v"]
        state = {}

        def qk_full(mc, nm):
            # batch-0 only: full-width [128,1024] psum from the (idle) pst
            # pool; evacuations split DVE/ACT (ACT idle before attention)
            def f():
                p = pstp.tile([128, N], FP32, tag="pst", name="pst")
                for nn in range(2):
                    for kc in range(4):
                        nc.tensor.matmul(
                            p[:, nn * 512 : (nn + 1) * 512],
                            wsl(nm, kc, mc),
                            xs(b, kc, nn * 512, (nn + 1) * 512),
                            start=(kc == 0),
                            stop=(kc == 3),
                        )
                if nm == "wq":
                    qpair = work.tile([128, N], BF16, tag="qpair", name="qpair", bufs=2)
                    state["qpair", mc] = qpair
                    nc.vector.tensor_copy(qpair[:, 0:512], p[:, 0:512])
                    nc.scalar.copy(qpair[:, 512:1024], p[:, 512:1024])
                else:
                    nc.vector.tensor_copy(kaug[2 * mc][0:64, :], p[0:64, :])
                    nc.scalar.copy(kaug[2 * mc + 1][0:64, :], p[64:128, :])
                    nc.vector.tensor_copy(kaug[2 * mc][64:128, :], oh_t[:])
                    nc.vector.tensor_copy(kaug[2 * mc + 1][64:128, :], oh_t[:])
            return f

        def qk_half(mc, nm, nn):
            def f():
                p = mmp.tile([128, 512], FP32, tag="mm", name="mm")
                for kc in range(4):
                    nc.tensor.matmul(
                        p[:],
                        wsl(nm, kc, mc),
                        xs(b, kc, nn * 512, (nn + 1) * 512),
                        start=(kc == 0),
                        stop=(kc == 3),
                    )
                if nm == "wq":
                    qpair = state.get(("qpair", mc))
                    if qpair is None:
                        qpair = work.tile([128, N], BF16, tag="qpair", name="qpair", bufs=2)
                        state["qpair", mc] = qpair
                    nc.vector.tensor_copy(qpair[:, nn * 512 : (nn + 1) * 512], p[:])
                else:
                    sl = slice(nn * 512, (nn + 1) * 512)
                    nc.vector.tensor_copy(kaug[2 * mc][0:64, sl], p[0:64, :])
                    nc.vector.tensor_copy(kaug[2 * mc + 1][0:64, sl], p[64:128, :])
                    if nn == 1:
                        nc.gpsimd.tensor_copy(kaug[2 * mc][64:128, :], oh_t[:])
                        nc.gpsimd.tensor_copy(kaug[2 * mc + 1][64:128, :], oh_t[:])
            return f

        def rel_half(mc, which, nn):
            def f():
                qpair = state["qpair", mc]
                p = mmp.tile([126, 512], FP32, tag="mm", name="mm")
                if which == "lh":
                    nc.tensor.matmul(
                        p[:],
                        rh_t[:],
                        qpair[:, nn * 512 : (nn + 1) * 512],
                        start=True,
                        stop=True,
                    )
                else:
                    qp = qpair[:]
                    ym = AP(qp.tensor, qp.offset + nn * 16, [[1024, 128], [1, 16], [32, 32]])
                    nc.tensor.matmul(p[:], rw_t[:], ym, start=True, stop=True)
                key = ("l" + which[1], mc)
                lt = state.get(key)
                if lt is None:
                    lt = work.tile([126, N], BF16, tag=f"l{which[1]}t", name=f"l{which[1]}t", bufs=2)
                    state[key] = lt
                    state[key + ("d",)] = dram.tile([126, N], BF16, tag=f"l{which[1]}td", name=f"l{which[1]}td")
                if b == 0 and nn == 1:
                    # batch-0 serial phase: ACT is idle, DVE is the pacer
                    nc.scalar.copy(lt[:, 512:1024], p[:])
                else:
                    nc.vector.tensor_copy(lt[:, nn * 512 : (nn + 1) * 512], p[:])
                if nn == 1:
                    nc.sync.dma_start(state[key + ("d",)][:], lt[:])
            return f

        def v_block(nb):
            def f():
                pv = mmp.tile([128, 512], FP32, tag="mm", name="mm")
                for kc in range(4):
                    nc.tensor.matmul(
                        pv[:],
                        xs(b, kc, nb * 128, (nb + 1) * 128),
                        w_t["wv"][:, kc * 512 : (kc + 1) * 512],
                        start=(kc == 0),
                        stop=(kc == 3),
                    )
                va = vaug[nb][:]
                vdst = AP(va.tensor, va.offset, [[520, 128], [65, 8], [1, 64]])
                pvs = pv[:]
                vsrc = AP(pvs.tensor, pvs.offset, [[512, 128], [64, 8], [1, 64]])
                nc.vector.tensor_copy(vdst, vsrc)
            return f

        def gathers(mc, hh):
            def f():
                # b1's SBUF->SBUF copies go on the (otherwise idle) GpSimd
                # engine to keep DVE headroom during the attention slots.
                cpy = nc.gpsimd.tensor_copy if b == 1 else nc.vector.tensor_copy
                h = 2 * mc + hh
                qpair = state["qpair", mc]
                cpy(qaug[h][0:64, :], qpair[hh * 64 : hh * 64 + 64, :])
                lhd = state["lh", mc, "d"][:]
                diag_h = AP(
                    lhd.tensor,
                    lhd.offset + (hh * 63 + 31) * N,
                    [[N, 32], [-(N - 32), 32], [1, 32]],
                )
                nc.sync.dma_start(
                    qaug[h][64:96, :].rearrange("p (a b) -> p a b", a=32), diag_h
                )
                awym = work.tile([32, N], BF16, tag="awym", name="awym")
                lwd = state["lw", mc, "d"][:]
                diag_w = AP(
                    lwd.tensor,
                    lwd.offset + (hh * 63 + 31) * N,
                    [[N, 32], [-(N - 32), 32], [1, 32]],
                )
                nc.sync.dma_start(
                    awym[:].rearrange("p (a b) -> p a b", a=32), diag_w
                )
                aw = awym[:]
                src = AP(aw.tensor, aw.offset, [[1024, 32], [1, 32], [32, 32]])
                cpy(
                    qaug[h][96:128, :].rearrange("p (a b) -> p a b", a=32), src
                )
            return f

        def tail_chunks(mc):
            return [
                rel_half(mc, "lh", 0),
                rel_half(mc, "lh", 1),
                rel_half(mc, "lw", 0),
                rel_half(mc, "lw", 1),
                gathers(mc, 0),
                gathers(mc, 1),
                v_block(2 * mc),
                v_block(2 * mc + 1),
            ]

        if b == 0:
            qks = [qk_full(mc, nm) for mc in range(4) for nm in ("wq", "wk")]
            tails = [f for mc in range(4) for f in tail_chunks(mc)]
            return qks, tails
        # batch 1, woven into attention slots at 4 chunks/slot. Order is
        # chosen so every chunk is EMITTED before the attention stream
        # reads its outputs (deadlines: vaug(*) before slot 9 = AV(1,0);
        # qaug/kaug(1,h) before slot 8+h). v_blocks depend only on x+wv,
        # so they are hoisted ahead of mc2/mc3.
        def qk_rel_g(mc):
            return [
                qk_half(mc, "wq", 0), qk_half(mc, "wq", 1),
                qk_half(mc, "wk", 0), qk_half(mc, "wk", 1),
                rel_half(mc, "lh", 0), rel_half(mc, "lh", 1),
                rel_half(mc, "lw", 0), rel_half(mc, "lw", 1),
                gathers(mc, 0), gathers(mc, 1),
            ]

        # gates spread the b1 projections across slots 1-12 so their DVE
        # evacs never cluster enough to back up the AV normalizes
        out = [(1, f) for f in qk_rel_g(0)]
        out += [(3, f) for f in qk_rel_g(1)]
        out += [(4, v_block(0)), (4, v_block(1))]
        out += [(5, v_block(nb)) for nb in range(2, 8)]
        out += [(8, f) for f in qk_rel_g(2)]
        out += [(11, f) for f in qk_rel_g(3)]
        return out


    def oproj_chunks(b):
        """O-projection. Batch 0: whole chunks gated after finish_av(0,7)
        (slot 9). Batch 1 necessarily tails the pipeline: the kc0-2
        accumulation (needs heads 0-5 only) runs from slot 15 / the av-tail,
        and only the kc3+bias+store part waits for the final normalize;
        bias adds go on ACT (idle at drain, Identity shares the exp set)."""
        oin = bt[b]["o"]
        chunks = []

        def po_full(mc, nn, oo):
            def f():
                po = mmp.tile([128, 512], FP32, tag="mm", name="mm")
                for kc in range(4):
                    nc.tensor.matmul(
                        po[:],
                        wsl("wo", kc, mc),
                        oin[kc][:, nn * 512 : (nn + 1) * 512],
                        start=(kc == 0),
                        stop=(kc == 3),
                    )
                nc.vector.tensor_add(
                    oo[:, nn * 512 : (nn + 1) * 512],
                    po[:],
                    bo_t[:, mc : mc + 1].broadcast_to((128, 512)),
                )
                nc.sync.dma_start(
                    y_out[b, mc * 128 : (mc + 1) * 128, nn * 512 : (nn + 1) * 512],
                    oo[:, nn * 512 : (nn + 1) * 512],
                )
            return f

        def po_partA(mc, st):
            # b1: kc0-2 accumulation for both halves in one pst-pool tile —
            # the pst pool is idle once the last S head is done (slot 15).
            def f():
                po = pstp.tile([128, N], FP32, tag="pst", name="pst")
                st["po", mc] = po
                for nn in range(2):
                    for kc in range(3):
                        nc.tensor.matmul(
                            po[:, nn * 512 : (nn + 1) * 512],
                            wsl("wo", kc, mc),
                            oin[kc][:, nn * 512 : (nn + 1) * 512],
                            start=(kc == 0),
                            stop=False,
                        )
            return f

        def po_partB(mc, oo, st):
            # final kc3 + bias on ACT (idle at drain) + store
            def f():
                po = st["po", mc]
                for nn in range(2):
                    nc.tensor.matmul(
                        po[:, nn * 512 : (nn + 1) * 512],
                        wsl("wo", 3, mc),
                        oin[3][:, nn * 512 : (nn + 1) * 512],
                        start=False,
                        stop=True,
                    )
                for nn in range(2):
                    nc.scalar.activation(
                        oo[:, nn * 512 : (nn + 1) * 512],
                        po[:, nn * 512 : (nn + 1) * 512],
                        mybir.ActivationFunctionType.Identity,
                        bias=bo_t[:, mc : mc + 1],
                        scale=1.0,
                    )
                    nc.sync.dma_start(
                        y_out[b, mc * 128 : (mc + 1) * 128, nn * 512 : (nn + 1) * 512],
                        oo[:, nn * 512 : (nn + 1) * 512],
                    )
            return f

        if b == 0:
            for mc in range(4):
                oo = work.tile([128, N], FP32, tag="oout", name="oout", bufs=2)
                chunks.append((10, po_full(mc, 0, oo)))
                chunks.append((10, po_full(mc, 1, oo)))
            return chunks
        st = {}
        oos = {mc: work.tile([128, N], FP32, tag="oout", name="oout", bufs=2) for mc in range(4)}
        chunks.append((17, po_partA(0, st)))
        chunks.append((17, po_partA(1, st)))
        chunks.append((17, po_partB(0, oos[0], st)))
        chunks.append((17, po_partB(1, oos[1], st)))
        chunks.append((17, po_partA(2, st)))
        chunks.append((17, po_partA(3, st)))
        chunks.append((17, po_partB(2, oos[2], st)))
        chunks.append((17, po_partB(3, oos[3], st)))
        return chunks

    def proj_chunks0_parts():
        return proj_chunks(0)

    # =================== emission ===================
    emit_x_load(0, cold=True)
    for kc in range(4):
        nc.sync.dma_start(
            w_t["wk"][:, kc * 512 : (kc + 1) * 512], wk_in[:, kc * 512 : (kc + 1) * 512]
        )
    for kc in range(4):
        nc.sync.dma_start(
            w_t["wv"][:, kc * 512 : (kc + 1) * 512], wv_in[:, kc * 512 : (kc + 1) * 512]
        )
    nc.sync.dma_start(oh_t[:], oh_in[:])
    nc.sync.dma_start(rh_t[:], rh_in[:])
    nc.sync.dma_start(rw_t[:], rw_in[:])
    nc.sync.dma_start(bo_t[:], bo_in[:].rearrange("(c p) one -> p (c one)", p=128))
    nc.sync.dma_start(id_t[:], id_in[:])

    # pre-warm the ACT exp table during the projection phase
    warm = work.tile([128, 4], FP32, tag="warm", name="warm", bufs=1)
    nc.scalar.activation(warm[:], bo_t[:], Exp, bias=0.0, scale=1.0)

    # batch 0 projections: coarse software pipeline, one mc ahead (qk of
    # mc+1 before tail of mc so PE isn't gated on tail's DVE evacs)
    g0, t0 = proj_chunks0_parts()
    order = g0[0:4] + t0[0:8] + g0[4:8] + t0[8:32]
    for ci, f in enumerate(order):
        f()
        if ci == 5:
            emit_vaug_ones(0)

    emit_x_load(1)
    nc.sync.dma_start(w_t["wo"][:], wo_in[:])
    emit_vaug_ones(1)

    filler = proj_chunks(1) + oproj_chunks(0) + oproj_chunks(1)

    

    def fill(slot, k=1):
        while k > 0 and filler and filler[0][0] <= slot:
            filler.pop(0)[1]()
            k -= 1

    # ---- unified attention pipeline: 16 head slots across both batches ----
    heads = [(b, h) for b in range(NB) for h in range(NH)]
    est_all = {}

    def emit_st_jb(bh, jb):
        b, h = bh
        pst = pstp.tile([128, N], FP32, tag="pst", name="pst")
        for nn in range(2):
            nc.tensor.matmul(
                pst[:, nn * 512 : (nn + 1) * 512],
                bt[b]["k"][h][:, jb * 128 : (jb + 1) * 128],
                bt[b]["q"][h][:, nn * 512 : (nn + 1) * 512],
                start=True,
                stop=True,
            )
        nc.scalar.activation(est_all[bh][jb][:], pst[:], Exp, bias=0.0, scale=0.125)

    qstate = {}

    def emit_av_qb(bh, qb):
        # Transposed AV: est chunks are the stationary operand (weight loads
        # are free on PE); the moving operand is V+ones (65 cols) instead of
        # est (1024 cols) -> 65*8 instead of 1024*8 PE cycles per (head, qb).
        # Four qb's share one [128, 4*65] PSUM tile (one bank): one batched
        # recip+mult per quad keeps the DVE work low and gives the av-tag
        # rotation ~4 phases of slack before the PE would block on it.
        b, h = bh
        qq, lo = divmod(qb, 4)
        if lo == 0:
            qstate[bh, qq] = avp.tile([128, 260], FP32, tag="av", name="pav")
        pav = qstate[bh, qq]
        for kb in range(8):
            nc.tensor.matmul(
                pav[:, lo * 65 : lo * 65 + 65],
                est_all[bh][kb][:, qb * 128 : (qb + 1) * 128],
                bt[b]["v"][kb][:, h * 65 : h * 65 + 65],
                start=(kb == 0),
                stop=(kb == 7),
            )
        if lo == 3:
            del qstate[bh, qq]
            pv = pav[:]
            rec = work.tile([128, 4], FP32, tag="rec", name="rec", bufs=4)
            nc.vector.reciprocal(
                rec[:], AP(pv.tensor, pv.offset + 64, [[260, 128], [65, 4]])
            )
            ot = bt[b]["ot"][qq][:]
            nc.vector.tensor_mul(
                AP(ot.tensor, ot.offset + h * 64, [[2048, 128], [512, 4], [1, 64]]),
                AP(pv.tensor, pv.offset, [[260, 128], [65, 4], [1, 64]]),
                rec[:].broadcast_to((128, 4, 64)),
            )

    def emit_tr(b, qb, kc):
        # PE transpose (via identity) of the normalized [pix, hd] chunk back
        # to [hd, pix] for the O-projection; shares the "av" PSUM tag.
        ptr = avp.tile([128, 128], BF16, tag="av", name="ptr")
        qq, lo = divmod(qb, 4)
        nc.tensor.transpose(
            ptr[:], bt[b]["ot"][qq][:, lo * 512 + kc * 128 : lo * 512 + (kc + 1) * 128], id_t[:]
        )
        nc.vector.tensor_copy(bt[b]["o"][kc][:, qb * 128 : (qb + 1) * 128], ptr[:])

    tr_queue = []

    def emit_slot(i, bh, prev):
        if bh is not None:
            est_all[bh] = [expp.tile([128, N], BF16, tag="expst", name="expst") for _ in range(8)]
        for k in range(8):
            if bh is not None:
                emit_st_jb(bh, k)
            if prev is not None:
                emit_av_qb(prev, k)
            if tr_queue:
                emit_tr(*tr_queue.pop(0))
            fill(i)
        # transposes for this slot's (odd) head trail by one slot: the quad
        # norms complete at phases 3/7, so trs can't interleave same-slot.
        if prev is not None and prev[1] % 2 == 1:
            tr_queue.extend((prev[0], qb, prev[1] // 2) for qb in range(8))
        fill(i, k=2)

    for i, bh in enumerate(heads):
        emit_slot(i, bh, heads[i - 1] if i > 0 else None)
    # tail: last head's AV back-to-back (interleaved qb order pre-allocates
    # both quad tiles early), O-proj kc0-2 accumulation overlapping the
    # norms, inline transposes, then the kc3+bias+store chain.
    b_t, h_t = heads[-1]
    for qb in (0, 4, 1, 5, 2, 6, 3, 7):
        emit_av_qb(heads[-1], qb)
    fill(17, k=2)  # po_partA(0), po_partA(1)
    for qb in range(8):
        emit_tr(b_t, qb, h_t // 2)
    fill(99, k=len(filler))


def _host_prep(w_q, w_k, w_v, w_o, b_o, rel_h, rel_w):
    perm = np.array([(c % 64) * 8 + c // 64 for c in range(C)])  # c' -> orig c
    oh = np.zeros((64, N), np.float32)
    j = np.arange(N)
    oh[j // HW, j] = 1.0
    oh[32 + j % HW, j] = 1.0
    rh2 = np.zeros((128, 126), np.float32)
    rh2[0:64, 0:63] = rel_h.T
    rh2[64:128, 63:126] = rel_h.T
    rw2 = np.zeros((128, 126), np.float32)
    rw2[0:64, 0:63] = rel_w.T
    rw2[64:128, 63:126] = rel_w.T
    import ml_dtypes

    bf = lambda a: np.ascontiguousarray(a).astype(ml_dtypes.bfloat16)

    def wpack(w):  # (C_in, C_out) -> (128, (kc, cout))
        return w.reshape(4, 128, C).transpose(1, 0, 2).reshape(128, 4 * C)

    return dict(
        wq=bf(wpack(w_q[perm, :].T)),
        wk=bf(wpack(w_k[perm, :].T)),
        wv=bf(wpack(w_v[perm, :].T)),
        wo=bf(wpack(w_o.T)),
        onehot=bf(oh),
        relh2=bf(rh2),
        relw2=bf(rw2),
        bo=np.ascontiguousarray(b_o.reshape(C, 1), dtype=np.float32),
        ident=bf(np.eye(128, dtype=np.float32)),
    )


_CACHE = {}


def _build_program():
    if "nc" in _CACHE:
        return _CACHE["nc"], _CACHE["names"]
    nc = bacc.Bacc("TRN2", target_bir_lowering=False, debug=False, num_devices=NCORES)
    specs = [
        ("x", (NB, C, N), BF16),
        ("wq", (128, 4 * C), BF16),
        ("wk", (128, 4 * C), BF16),
        ("wv", (128, 4 * C), BF16),
        ("wo", (128, 4 * C), BF16),
        ("onehot", (64, N), BF16),
        ("relh2", (128, 126), BF16),
        ("relw2", (128, 126), BF16),
        ("bo", (C, 1), FP32),
        ("ident", (128, 128), BF16),
    ]
    in_aps = [nc.dram_tensor(nm, list(shape), dt, kind="ExternalInput").ap() for nm, shape, dt in specs]
    out_ap = nc.dram_tensor("y", [NB, C, N], FP32, kind="ExternalOutput").ap()
    with tile.TileContext(nc) as tc:
        with ExitStack() as ctx:
            _build_body(ctx, tc, [out_ap], in_aps, NB)
    nc.compile()
    _CACHE["nc"] = nc
    _CACHE["names"] = [s[0] for s in specs]
    return nc, _CACHE["names"]


def _run(inputs, trace=False, tmpdir=None):
    import ml_dtypes

    x = np.asarray(inputs["x"], dtype=np.float32)
    cst = _host_prep(
        np.asarray(inputs["w_q"], np.float32),
        np.asarray(inputs["w_k"], np.float32),
        np.asarray(inputs["w_v"], np.float32),
        np.asarray(inputs["w_o"], np.float32),
        np.asarray(inputs["b_o"], np.float32),
        np.asarray(inputs["rel_h"], np.float32),
        np.asarray(inputs["rel_w"], np.float32),
    )
    nc, _ = _build_program()
    xb = np.ascontiguousarray(x.reshape(B, C, N)).astype(ml_dtypes.bfloat16)
    in_maps = []
    for c in range(NCORES):
        m = dict(cst)
        m["x"] = np.ascontiguousarray(xb[c * NB : (c + 1) * NB])
        in_maps.append(m)
    res = run_bass_kernel_spmd(
        nc, in_maps, core_ids=list(range(NCORES)), trace=trace, tmpdir=tmpdir
    )
    out = np.empty((B, C, HW, HW), np.float32)
    for c in range(NCORES):
        out[c * NB : (c + 1) * NB] = res.results[c]["y"].reshape(NB, C, HW, HW)
    return out, res


def kernel(**inputs):
    out, _ = _run(inputs, trace=False)
    return out



# revision 31
# speedup vs baseline: 1.0109x; 1.0070x over previous
"""BottleNeck-MHSA (B=16, C=512, H=W=32, NH=8, DK=64) on 8 Trainium2 cores.

Sharding: pure data-parallel over batch (2 batches per core), no collectives.

Design (per core, NB=2 batches):
- Augmented-contraction trick: S'^T = [kT|onehots]^T @ [qT|ahT|awT] folds the
  content-dependent rel-pos bias into the energy matmul (K=128 contraction,
  same PE cost as the bare K=64 energy matmul). ahT/awT are content-dependent
  diagonal gathers of rel_h@q / rel_w@q via a DRAM round-trip 3D-strided DMA.
- Softmax skips max-subtraction (logits bounded); exp on ACT with the
  1/sqrt(DK) scale fused; the denominator comes replicated out of the AV
  matmul via ones-columns in the V lhsT; normalization deferred past AV
  (DVE reciprocal+multiply into the O-projection input).
- All SBUF operands bf16 (converted host-side); PSUM fp32. ACT is reserved
  for exp; PSUM evacuations and bf16 copies on DVE (2x/4x perf modes);
  GpSimd only memsets.
- Unified 16-head software pipeline across both batches: per head slot the
  previous head's AV runs as two half-width [128,512] accumulations whose
  normalize fires mid-slot (decouples the next AV from DVE latency), woven
  with this head's S^T/exp at the ACT exp cadence, plus gated filler chunks:
  batch 1's projections and both batches' O-projections, split into ~2-4
  matmul closures. Gates keep PE emission topological (FIFO-safe).
- PSUM (8 banks): pst 2x[128,1024] (4) + proj 2x[128,512] (2) +
  av 2x[128,512] (2).
- TimelineSim: ~194.6 us/core (PE-busy floor ~171 us); rel err ~5.3e-3.
"""

from contextlib import ExitStack

import numpy as np

import concourse.bass as bass
import concourse.tile as tile
from concourse import bacc, mybir
from concourse.ap import AP
from concourse.bass_utils import run_bass_kernel_spmd

FP32 = mybir.dt.float32
BF16 = mybir.dt.bfloat16
Exp = mybir.ActivationFunctionType.Exp

B = 16
C = 512
N = 1024
NH = 8
DK = 64
HW = 32
NCORES = 8
NB = B // NCORES  # batches per core


def _build_body(ctx: ExitStack, tc: tile.TileContext, outs, ins, NB: int):
    nc = tc.nc
    x_in, wq_in, wk_in, wv_in, wo_in, oh_in, rh_in, rw_in, bo_in, id_in = ins
    y_out = outs[0]

    consts = ctx.enter_context(tc.tile_pool(name="consts", bufs=1))
    dbl = ctx.enter_context(tc.tile_pool(name="dbl", bufs=2))
    work = ctx.enter_context(tc.tile_pool(name="work", bufs=2))
    # est tiles: two full heads of 8 tiles each — AV(h) reads all 8 of head
    # h's tiles in every qb-phase of slot h+1, while S/exp(h+1) writes its 8.
    expp = ctx.enter_context(tc.tile_pool(name="expp", bufs=16))
    pstp = ctx.enter_context(tc.tile_pool(name="pstp", bufs=2, space="PSUM"))
    mmp = ctx.enter_context(tc.tile_pool(name="mmp", bufs=2, space="PSUM"))
    avp = ctx.enter_context(tc.tile_pool(name="avp", bufs=2, space="PSUM"))
    dram = ctx.enter_context(tc.tile_pool(name="dram", bufs=2, space="DRAM"))

    # ---------------- tiles ----------------
    # weight layout: [128 (cin within kc-block), (kc, cout_mc)]; slice for
    # (kc, mc) = cols [kc*512 + mc*128, +128)
    w_t = {nm: consts.tile([128, 4 * C], BF16, tag=nm, name=nm) for nm in ("wq", "wk", "wv", "wo")}

    def wsl(nm, kc, mc):
        return w_t[nm][:, kc * 512 + mc * 128 : kc * 512 + (mc + 1) * 128]

    oh_t = consts.tile([64, N], BF16, tag="onehot", name="onehot")
    rh_t = consts.tile([128, 126], BF16, tag="relh2", name="relh2")
    rw_t = consts.tile([128, 126], BF16, tag="relw2", name="relw2")
    id_t = consts.tile([128, 128], BF16, tag="ident", name="ident")
    bo_t = consts.tile([128, 4], FP32, tag="bo", name="bo")

    bt = {}
    for b in range(NB):
        bt[b] = dict(
            # x layout: [128 (cin within kc), (kc, n)]
            x=dbl.tile([128, 4 * N], BF16, tag="x", name=f"x_{b}"),
            q=[dbl.tile([128, N], BF16, tag=f"qaug{h}", name=f"qaug{h}_{b}") for h in range(NH)],
            k=[dbl.tile([128, N], BF16, tag=f"kaug{h}", name=f"kaug{h}_{b}") for h in range(NH)],
            # vaug[nb]: AV moving operand; head h at cols [65h,65h+64)=V, col 65h+64=ones
            # (denominator column for the transposed AV)
            v=[dbl.tile([128, 520], BF16, tag=f"vaug{nb}", name=f"vaug{nb}_{b}") for nb in range(8)],
            o=[dbl.tile([128, N], BF16, tag=f"oin{kc}", name=f"oin{kc}_{b}") for kc in range(4)],
            # ot[qq]: normalized attention out per qb-quad,
            # [128 pix, (4 qb, 8 h, 64 d)] bf16
            ot=[dbl.tile([128, 2048], BF16, tag=f"ot{qq}", name=f"ot{qq}_{b}") for qq in range(2)],
        )

    def xs(b, kc, lo, hi):
        return bt[b]["x"][:, kc * N + lo : kc * N + hi]

    def emit_x_load(b, cold=False):
        if not cold:
            for kc in range(4):
                nc.sync.dma_start(
                    bt[b]["x"][:, kc * N : (kc + 1) * N],
                    x_in[b, kc * 128 : (kc + 1) * 128, :],
                )
            return
        # cold start: (wq slice, x chunk) pairs so each kc matmul's inputs
        # arrive together; nn0 halves first to match matmul order
        for kc in range(4):
            nc.sync.dma_start(
                w_t["wq"][:, kc * 512 : (kc + 1) * 512],
                wq_in[:, kc * 512 : (kc + 1) * 512],
            )
            nc.sync.dma_start(
                bt[b]["x"][:, kc * N : kc * N + 512],
                x_in[b, kc * 128 : (kc + 1) * 128, 0:512],
            )
        for kc in range(4):
            nc.sync.dma_start(
                bt[b]["x"][:, kc * N + 512 : kc * N + 1024],
                x_in[b, kc * 128 : (kc + 1) * 128, 512:1024],
            )

    def emit_vaug_ones(b):
        for nb in range(8):
            va = bt[b]["v"][nb][:]
            ones_ap = AP(va.tensor, va.offset + 64, [[520, 128], [65, 8]])
            nc.gpsimd.memset(ones_ap, 1.0)

    # ---- projection chunk builders (each chunk ~1-4 matmuls + evac work) ----
    def proj_chunks(b):
        """List of (gate_slot, closure); call in list order (dep-ordered)."""
        qaug, kaug, vaug = bt[b]["q"], bt[b]["k"], bt[b]["v"]
        state = {}

        def qk_full(mc, nm):
            # batch-0 only: full-width [128,1024] psum from the (idle) pst
            # pool; evacuations split DVE/ACT (ACT idle before attention)
            def f():
                p = pstp.tile([128, N], FP32, tag="pst", name="pst")
                for nn in range(2):
                    for kc in range(4):
                        nc.tensor.matmul(
                            p[:, nn * 512 : (nn + 1) * 512],
                            wsl(nm, kc, mc),
                            xs(b, kc, nn * 512, (nn + 1) * 512),
                            start=(kc == 0),
                            stop=(kc == 3),
                        )
                if nm == "wq":
                    qpair = work.tile([128, N], BF16, tag="qpair", name="qpair", bufs=2)
                    state["qpair", mc] = qpair
                    nc.vector.tensor_copy(qpair[:, 0:512], p[:, 0:512])
                    nc.scalar.copy(qpair[:, 512:1024], p[:, 512:1024])
                else:
                    nc.vector.tensor_copy(kaug[2 * mc][0:64, :], p[0:64, :])
                    nc.scalar.copy(kaug[2 * mc + 1][0:64, :], p[64:128, :])
                    nc.vector.tensor_copy(kaug[2 * mc][64:128, :], oh_t[:])
                    nc.vector.tensor_copy(kaug[2 * mc + 1][64:128, :], oh_t[:])
            return f

        def qk_half(mc, nm, nn):
            def f():
                p = mmp.tile([128, 512], FP32, tag="mm", name="mm")
                for kc in range(4):
                    nc.tensor.matmul(
                        p[:],
                        wsl(nm, kc, mc),
                        xs(b, kc, nn * 512, (nn + 1) * 512),
                        start=(kc == 0),
                        stop=(kc == 3),
                    )
                if nm == "wq":
                    qpair = state.get(("qpair", mc))
                    if qpair is None:
                        qpair = work.tile([128, N], BF16, tag="qpair", name="qpair", bufs=2)
                        state["qpair", mc] = qpair
                    nc.vector.tensor_copy(qpair[:, nn * 512 : (nn + 1) * 512], p[:])
                else:
                    sl = slice(nn * 512, (nn + 1) * 512)
                    nc.vector.tensor_copy(kaug[2 * mc][0:64, sl], p[0:64, :])
                    nc.vector.tensor_copy(kaug[2 * mc + 1][0:64, sl], p[64:128, :])
                    if nn == 1:
                        nc.gpsimd.tensor_copy(kaug[2 * mc][64:128, :], oh_t[:])
                        nc.gpsimd.tensor_copy(kaug[2 * mc + 1][64:128, :], oh_t[:])
            return f

        def rel_half(mc, which, nn):
            def f():
                qpair = state["qpair", mc]
                p = mmp.tile([126, 512], FP32, tag="mm", name="mm")
                if which == "lh":
                    nc.tensor.matmul(
                        p[:],
                        rh_t[:],
                        qpair[:, nn * 512 : (nn + 1) * 512],
                        start=True,
                        stop=True,
                    )
                else:
                    qp = qpair[:]
                    ym = AP(qp.tensor, qp.offset + nn * 16, [[1024, 128], [1, 16], [32, 32]])
                    nc.tensor.matmul(p[:], rw_t[:], ym, start=True, stop=True)
                key = ("l" + which[1], mc)
                lt = state.get(key)
                if lt is None:
                    lt = work.tile([126, N], BF16, tag=f"l{which[1]}t", name=f"l{which[1]}t", bufs=2)
                    state[key] = lt
                    state[key + ("d",)] = dram.tile([126, N], BF16, tag=f"l{which[1]}td", name=f"l{which[1]}td")
                if b == 0 and nn == 1:
                    # batch-0 serial phase: ACT is idle, DVE is the pacer
                    nc.scalar.copy(lt[:, 512:1024], p[:])
                else:
                    nc.vector.tensor_copy(lt[:, nn * 512 : (nn + 1) * 512], p[:])
                if nn == 1:
                    nc.sync.dma_start(state[key + ("d",)][:], lt[:])
            return f

        def v_block(nb):
            def f():
                pv = mmp.tile([128, 512], FP32, tag="mm", name="mm")
                for kc in range(4):
                    nc.tensor.matmul(
                        pv[:],
                        xs(b, kc, nb * 128, (nb + 1) * 128),
                        w_t["wv"][:, kc * 512 : (kc + 1) * 512],
                        start=(kc == 0),
                        stop=(kc == 3),
                    )
                va = vaug[nb][:]
                vdst = AP(va.tensor, va.offset, [[520, 128], [65, 8], [1, 64]])
                pvs = pv[:]
                vsrc = AP(pvs.tensor, pvs.offset, [[512, 128], [64, 8], [1, 64]])
                nc.vector.tensor_copy(vdst, vsrc)
            return f

        def gathers(mc, hh):
            def f():
                # b1's SBUF->SBUF copies go on the (otherwise idle) GpSimd
                # engine to keep DVE headroom during the attention slots.
                cpy = nc.gpsimd.tensor_copy if b == 1 else nc.vector.tensor_copy
                h = 2 * mc + hh
                qpair = state["qpair", mc]
                cpy(qaug[h][0:64, :], qpair[hh * 64 : hh * 64 + 64, :])
                lhd = state["lh", mc, "d"][:]
                diag_h = AP(
                    lhd.tensor,
                    lhd.offset + (hh * 63 + 31) * N,
                    [[N, 32], [-(N - 32), 32], [1, 32]],
                )
                nc.sync.dma_start(
                    qaug[h][64:96, :].rearrange("p (a b) -> p a b", a=32), diag_h
                )
                awym = work.tile([32, N], BF16, tag="awym", name="awym")
                lwd = state["lw", mc, "d"][:]
                diag_w = AP(
                    lwd.tensor,
                    lwd.offset + (hh * 63 + 31) * N,
                    [[N, 32], [-(N - 32), 32], [1, 32]],
                )
                nc.sync.dma_start(
                    awym[:].rearrange("p (a b) -> p a b", a=32), diag_w
                )
                aw = awym[:]
                src = AP(aw.tensor, aw.offset, [[1024, 32], [1, 32], [32, 32]])
                cpy(
                    qaug[h][96:128, :].rearrange("p (a b) -> p a b", a=32), src
                )
            return f

        def tail_chunks(mc):
            return [
                rel_half(mc, "lh", 0),
                rel_half(mc, "lh", 1),
                rel_half(mc, "lw", 0),
                rel_half(mc, "lw", 1),
                gathers(mc, 0),
                gathers(mc, 1),
                v_block(2 * mc),
                v_block(2 * mc + 1),
            ]

        if b == 0:
            qks = [qk_full(mc, nm) for mc in range(4) for nm in ("wq", "wk")]
            tails = [f for mc in range(4) for f in tail_chunks(mc)]
            return qks, tails
        # batch 1, woven into attention slots at 4 chunks/slot. Order is
        # chosen so every chunk is EMITTED before the attention stream
        # reads its outputs (deadlines: vaug(*) before slot 9 = AV(1,0);
        # qaug/kaug(1,h) before slot 8+h). v_blocks depend only on x+wv,
        # so they are hoisted ahead of mc2/mc3.
        def qk_rel_g(mc):
            return [
                qk_half(mc, "wq", 0), qk_half(mc, "wq", 1),
                qk_half(mc, "wk", 0), qk_half(mc, "wk", 1),
                rel_half(mc, "lh", 0), rel_half(mc, "lh", 1),
                rel_half(mc, "lw", 0), rel_half(mc, "lw", 1),
                gathers(mc, 0), gathers(mc, 1),
            ]

        # gates spread the b1 projections across slots 1-12 so their DVE
        # evacs never cluster enough to back up the AV normalizes
        out = [(1, f) for f in qk_rel_g(0)]
        out += [(3, f) for f in qk_rel_g(1)]
        out += [(4, v_block(0)), (4, v_block(1))]
        out += [(5, v_block(nb)) for nb in range(2, 8)]
        out += [(8, f) for f in qk_rel_g(2)]
        out += [(11, f) for f in qk_rel_g(3)]
        return out


    def oproj_chunks(b):
        """O-projection. Batch 0: whole chunks gated after finish_av(0,7)
        (slot 9). Batch 1 necessarily tails the pipeline: the kc0-2
        accumulation (needs heads 0-5 only) runs from slot 15 / the av-tail,
        and only the kc3+bias+store part waits for the final normalize;
        bias adds go on ACT (idle at drain, Identity shares the exp set)."""
        oin = bt[b]["o"]
        chunks = []

        def po_full(mc, nn):
            def f():
                po = mmp.tile([128, 512], FP32, tag="mm", name="mm")
                for kc in range(4):
                    nc.tensor.matmul(
                        po[:],
                        wsl("wo", kc, mc),
                        oin[kc][:, nn * 512 : (nn + 1) * 512],
                        start=(kc == 0),
                        stop=(kc == 3),
                    )
                oo = work.tile([128, 512], FP32, tag="oout", name="oout", bufs=4)
                nc.vector.tensor_add(
                    oo[:], po[:], bo_t[:, mc : mc + 1].broadcast_to((128, 512))
                )
                nc.sync.dma_start(
                    y_out[b, mc * 128 : (mc + 1) * 128, nn * 512 : (nn + 1) * 512],
                    oo[:],
                )
            return f

        def po_partA(mc, st, pool):
            # b1: bias + kc0-2 accumulation for both halves; pst pool is idle
            # once the last S head is done, mm is idle in the tail.
            def f():
                if pool is pstp:
                    po = pstp.tile([128, N], FP32, tag="pst", name="pst")
                    halves = [po[:, 0:512], po[:, 512:1024]]
                else:
                    halves = [
                        mmp.tile([128, 512], FP32, tag="mm", name="mm")[:]
                        for _ in range(2)
                    ]
                st["po", mc] = halves
                for nn in range(2):
                    for kc in range(3):
                        nc.tensor.matmul(
                            halves[nn],
                            wsl("wo", kc, mc),
                            oin[kc][:, nn * 512 : (nn + 1) * 512],
                            start=(kc == 0),
                            stop=False,
                        )
            return f

        def po_partB(mc, st):
            # final kc3 + bias evac on ACT (idle in the tail) + store
            def f():
                halves = st["po", mc]
                for nn in range(2):
                    nc.tensor.matmul(
                        halves[nn],
                        wsl("wo", 3, mc),
                        oin[3][:, nn * 512 : (nn + 1) * 512],
                        start=False,
                        stop=True,
                    )
                for nn in range(2):
                    oo = work.tile([128, 512], FP32, tag="oout", name="oout", bufs=4)
                    nc.scalar.activation(
                        oo[:],
                        halves[nn],
                        mybir.ActivationFunctionType.Identity,
                        bias=bo_t[:, mc : mc + 1],
                        scale=1.0,
                    )
                    nc.sync.dma_start(
                        y_out[b, mc * 128 : (mc + 1) * 128, nn * 512 : (nn + 1) * 512],
                        oo[:],
                    )
            return f

        if b == 0:
            for mc in range(4):
                chunks.append((10, po_full(mc, 0)))
                chunks.append((10, po_full(mc, 1)))
            return chunks
        st = {}
        chunks.append((17, po_partA(0, st, pstp)))
        chunks.append((17, po_partA(1, st, pstp)))
        chunks.append((17, po_partB(0, st)))
        chunks.append((17, po_partB(1, st)))
        chunks.append((17, po_partA(2, st, mmp)))
        chunks.append((17, po_partA(3, st, pstp)))
        chunks.append((17, po_partB(2, st)))
        chunks.append((17, po_partB(3, st)))
        return chunks

    def proj_chunks0_parts():
        return proj_chunks(0)

    # =================== emission ===================
    emit_x_load(0, cold=True)
    for kc in range(4):
        nc.sync.dma_start(
            w_t["wk"][:, kc * 512 : (kc + 1) * 512], wk_in[:, kc * 512 : (kc + 1) * 512]
        )
    for kc in range(4):
        nc.sync.dma_start(
            w_t["wv"][:, kc * 512 : (kc + 1) * 512], wv_in[:, kc * 512 : (kc + 1) * 512]
        )
    nc.sync.dma_start(oh_t[:], oh_in[:])
    nc.sync.dma_start(rh_t[:], rh_in[:])
    nc.sync.dma_start(rw_t[:], rw_in[:])
    nc.sync.dma_start(bo_t[:], bo_in[:].rearrange("(c p) one -> p (c one)", p=128))
    nc.sync.dma_start(id_t[:], id_in[:])

    # pre-warm the ACT exp table during the projection phase
    warm = work.tile([128, 4], FP32, tag="warm", name="warm", bufs=1)
    nc.scalar.activation(warm[:], id_t[:, 0:4], Exp, bias=0.0, scale=1.0)

    # batch 0 projections: coarse software pipeline, one mc ahead (qk of
    # mc+1 before tail of mc so PE isn't gated on tail's DVE evacs)
    g0, t0 = proj_chunks0_parts()
    order = g0[0:4] + t0[0:8] + g0[4:8] + t0[8:32]
    for ci, f in enumerate(order):
        f()
        if ci == 5:
            emit_vaug_ones(0)

    emit_x_load(1)
    nc.sync.dma_start(w_t["wo"][:], wo_in[:])
    emit_vaug_ones(1)

    filler = proj_chunks(1) + oproj_chunks(0) + oproj_chunks(1)

    

    def fill(slot, k=1):
        while k > 0 and filler and filler[0][0] <= slot:
            filler.pop(0)[1]()
            k -= 1

    # ---- unified attention pipeline: 16 head slots across both batches ----
    heads = [(b, h) for b in range(NB) for h in range(NH)]
    est_all = {}

    def emit_st_jb(bh, jb):
        b, h = bh
        pst = pstp.tile([128, N], FP32, tag="pst", name="pst")
        for nn in range(2):
            nc.tensor.matmul(
                pst[:, nn * 512 : (nn + 1) * 512],
                bt[b]["k"][h][:, jb * 128 : (jb + 1) * 128],
                bt[b]["q"][h][:, nn * 512 : (nn + 1) * 512],
                start=True,
                stop=True,
            )
        nc.scalar.activation(est_all[bh][jb][:], pst[:], Exp, bias=0.0, scale=0.125)

    qstate = {}

    def emit_av_qb(bh, qb):
        # Transposed AV: est chunks are the stationary operand (weight loads
        # are free on PE); the moving operand is V+ones (65 cols) instead of
        # est (1024 cols) -> 65*8 instead of 1024*8 PE cycles per (head, qb).
        # Four qb's share one [128, 4*65] PSUM tile (one bank): one batched
        # recip+mult per quad keeps the DVE work low and gives the av-tag
        # rotation ~4 phases of slack before the PE would block on it.
        b, h = bh
        qq, lo = divmod(qb, 4)
        if lo == 0:
            qstate[bh, qq] = avp.tile([128, 260], FP32, tag="av", name="pav")
        pav = qstate[bh, qq]
        for kb in range(8):
            nc.tensor.matmul(
                pav[:, lo * 65 : lo * 65 + 65],
                est_all[bh][kb][:, qb * 128 : (qb + 1) * 128],
                bt[b]["v"][kb][:, h * 65 : h * 65 + 65],
                start=(kb == 0),
                stop=(kb == 7),
            )
        if lo == 3:
            del qstate[bh, qq]
            pv = pav[:]
            rec = work.tile([128, 4], FP32, tag="rec", name="rec", bufs=4)
            nc.vector.reciprocal(
                rec[:], AP(pv.tensor, pv.offset + 64, [[260, 128], [65, 4]])
            )
            ot = bt[b]["ot"][qq][:]
            nc.vector.tensor_mul(
                AP(ot.tensor, ot.offset + h * 64, [[2048, 128], [512, 4], [1, 64]]),
                AP(pv.tensor, pv.offset, [[260, 128], [65, 4], [1, 64]]),
                rec[:].broadcast_to((128, 4, 64)),
            )

    def emit_tr(b, qb, kc):
        # PE transpose (via identity) of the normalized [pix, hd] chunk back
        # to [hd, pix] for the O-projection; shares the "av" PSUM tag.
        ptr = avp.tile([128, 128], BF16, tag="av", name="ptr")
        qq, lo = divmod(qb, 4)
        nc.tensor.transpose(
            ptr[:], bt[b]["ot"][qq][:, lo * 512 + kc * 128 : lo * 512 + (kc + 1) * 128], id_t[:]
        )
        nc.vector.tensor_copy(bt[b]["o"][kc][:, qb * 128 : (qb + 1) * 128], ptr[:])

    tr_queue = []

    def emit_slot(i, bh, prev):
        if bh is not None:
            est_all[bh] = [expp.tile([128, N], BF16, tag="expst", name="expst") for _ in range(8)]
        for k in range(8):
            if bh is not None:
                emit_st_jb(bh, k)
            if prev is not None:
                emit_av_qb(prev, k)
            if tr_queue:
                emit_tr(*tr_queue.pop(0))
            fill(i)
        # transposes for this slot's (odd) head trail by one slot: the quad
        # norms complete at phases 3/7, so trs can't interleave same-slot.
        if prev is not None and prev[1] % 2 == 1:
            tr_queue.extend((prev[0], qb, prev[1] // 2) for qb in range(8))
        fill(i, k=2)

    for i, bh in enumerate(heads):
        emit_slot(i, bh, heads[i - 1] if i > 0 else None)
    # tail: last head's AV back-to-back (interleaved qb order pre-allocates
    # both quad tiles early), O-proj kc0-2 accumulation overlapping the
    # norms, inline transposes, then the kc3+bias+store chain.
    b_t, h_t = heads[-1]
    for qb in (0, 4, 1, 5, 2, 6, 3, 7):
        emit_av_qb(heads[-1], qb)
    fill(17, k=2)  # po_partA(0), po_partA(1)
    for qb in range(8):
        emit_tr(b_t, qb, h_t // 2)
    fill(99, k=len(filler))


def _host_prep(w_q, w_k, w_v, w_o, b_o, rel_h, rel_w):
    perm = np.array([(c % 64) * 8 + c // 64 for c in range(C)])  # c' -> orig c
    oh = np.zeros((64, N), np.float32)
    j = np.arange(N)
    oh[j // HW, j] = 1.0
    oh[32 + j % HW, j] = 1.0
    rh2 = np.zeros((128, 126), np.float32)
    rh2[0:64, 0:63] = rel_h.T
    rh2[64:128, 63:126] = rel_h.T
    rw2 = np.zeros((128, 126), np.float32)
    rw2[0:64, 0:63] = rel_w.T
    rw2[64:128, 63:126] = rel_w.T
    import ml_dtypes

    bf = lambda a: np.ascontiguousarray(a).astype(ml_dtypes.bfloat16)

    def wpack(w):  # (C_in, C_out) -> (128, (kc, cout))
        return w.reshape(4, 128, C).transpose(1, 0, 2).reshape(128, 4 * C)

    return dict(
        wq=bf(wpack(w_q[perm, :].T)),
        wk=bf(wpack(w_k[perm, :].T)),
        wv=bf(wpack(w_v[perm, :].T)),
        wo=bf(wpack(w_o.T)),
        onehot=bf(oh),
        relh2=bf(rh2),
        relw2=bf(rw2),
        bo=np.ascontiguousarray(b_o.reshape(C, 1), dtype=np.float32),
        ident=bf(np.eye(128, dtype=np.float32)),
    )


_CACHE = {}


def _build_program():
    if "nc" in _CACHE:
        return _CACHE["nc"], _CACHE["names"]
    nc = bacc.Bacc("TRN2", target_bir_lowering=False, debug=False, num_devices=NCORES)
    specs = [
        ("x", (NB, C, N), BF16),
        ("wq", (128, 4 * C), BF16),
        ("wk", (128, 4 * C), BF16),
        ("wv", (128, 4 * C), BF16),
        ("wo", (128, 4 * C), BF16),
        ("onehot", (64, N), BF16),
        ("relh2", (128, 126), BF16),
        ("relw2", (128, 126), BF16),
        ("bo", (C, 1), FP32),
        ("ident", (128, 128), BF16),
    ]
    in_aps = [nc.dram_tensor(nm, list(shape), dt, kind="ExternalInput").ap() for nm, shape, dt in specs]
    out_ap = nc.dram_tensor("y", [NB, C, N], FP32, kind="ExternalOutput").ap()
    with tile.TileContext(nc) as tc:
        with ExitStack() as ctx:
            _build_body(ctx, tc, [out_ap], in_aps, NB)
    nc.compile()
    _CACHE["nc"] = nc
    _CACHE["names"] = [s[0] for s in specs]
    return nc, _CACHE["names"]


def _run(inputs, trace=False, tmpdir=None):
    import ml_dtypes

    x = np.asarray(inputs["x"], dtype=np.float32)
    cst = _host_prep(
        np.asarray(inputs["w_q"], np.float32),
        np.asarray(inputs["w_k"], np.float32),
        np.asarray(inputs["w_v"], np.float32),
        np.asarray(inputs["w_o"], np.float32),
        np.asarray(inputs["b_o"], np.float32),
        np.asarray(inputs["rel_h"], np.float32),
        np.asarray(inputs["rel_w"], np.float32),
    )
    nc, _ = _build_program()
    xb = np.ascontiguousarray(x.reshape(B, C, N)).astype(ml_dtypes.bfloat16)
    in_maps = []
    for c in range(NCORES):
        m = dict(cst)
        m["x"] = np.ascontiguousarray(xb[c * NB : (c + 1) * NB])
        in_maps.append(m)
    res = run_bass_kernel_spmd(
        nc, in_maps, core_ids=list(range(NCORES)), trace=trace, tmpdir=tmpdir
    )
    out = np.empty((B, C, HW, HW), np.float32)
    for c in range(NCORES):
        out[c * NB : (c + 1) * NB] = res.results[c]["y"].reshape(NB, C, HW, HW)
    return out, res


def kernel(**inputs):
    out, _ = _run(inputs, trace=False)
    return out

